# revision 66
# baseline (speedup 1.0000x reference)
"""Trainium2 Bass kernel for causal multi-head attention with RoPE.

Model: B=2, T=2048, C=2048, H=16 heads, D=128 head_dim.
  qkv = x @ w_qkv ; q,k rotary-embedded ; causal softmax attention ; out @ w_out.

Sharding: tensor-parallel over heads. 16 heads / 8 cores = 2 heads per core.
Each core gets w_qkv columns and w_out rows for its 2 heads, computes a full
(B*T, C) partial output projection (bf16), and the host sums the 8 partials.

Mixed precision, chosen so the max-relative-error metric (dominated by early
tokens, whose concentrated softmax makes outputs large and error-proportional)
stays well under budget while late tokens ride cheap fp8:
  - token block 0 of each batch: bf16 x / w_qkv projections.
  - token blocks 1-3: fp8-e4m3 x / w_qkv with DoubleRow matmuls (256-deep
    contraction, half the PE instructions).  w_qkv is pre-scaled by WSCALE to
    clear fp8's denormal range; the scale is undone for free in the cos/sin
    tables (q,k) and the v-copy activation scale.
  - attention (scores/exp/out) in bf16; fp32 PSUM accumulation everywhere.
  - output projection: tq block 0 in bf16; blocks 1-3 contract both local
    heads in one fp8 DoubleRow matmul against WSCALE-scaled w_out, unscaled
    during the PSUM->SBUF copy.

Per-core dataflow:
  - x is fed pre-transposed (xT, [C, B*T]) so the C contraction sits on
    partitions.  qT/kT come out of the projection directly in [D, T] layout
    (D on partitions), v in natural [T, D] layout.
  - RoPE on DVE in [D, T] layout: rot_half is a partition-half swap done with
    two ACT copies, the sign folded into the sin table host-side.
  - scores computed transposed (sT[tk, tq] = kT.T @ qT); the causal mask
    (fully-masked columns AND the diagonal triangle, one composite constant
    sliced per diagonal position) is added on PE via an identity x mask
    matmul into the same PSUM bank, so every exp is one uniform full-width
    ACT op (masked entries underflow to zero) with 1/sqrt(D) folded in.
  - out_un[d, tq] accumulates v.T @ expT on PE; softmax denominators via a
    ones-column matmul over DVE-summed quads of e tiles.
  - normalization: fast approximate reciprocal of the colsum row on DVE,
    gpsimd partition-broadcast, one DVE multiply.
  - projection units are interleaved into the attention loops (PE backfill
    while ACT exp is the rate limiter) and stream to HBM as bf16 partials.
"""

import numpy as np
import ml_dtypes

import concourse.bass as bass
import concourse.tile as tile
import concourse.mybir as mybir
from concourse import bacc
from concourse.bass import ds
from concourse.bass_utils import run_bass_kernel_spmd

B, T, C, H, D = 2, 2048, 2048, 16, 128
NCORES = 8
HPC = H // NCORES  # heads per core = 2
S = B * T  # 4096 tokens
NBLK = T // 512  # 4 tq blocks of 512 per batch
NCT = C // 128  # 16 contraction tiles for the qkv projection
NTK = T // 128  # 16 tk tiles per batch
F32 = mybir.dt.float32
BF16 = mybir.dt.bfloat16
FP8 = mybir.dt.float8e4
WSCALE = 64.0  # w_qkv pre-scaled by this (fp8 denormal avoidance); undone in
               # the cos/sin tables (q,k) and the v-copy activation scale
EXP_SCALE = float(D) ** -0.5
NEG = -1.0e30

_CACHE = {}


def build_nc(reps=1):
    nc = bacc.Bacc("TRN2", target_bir_lowering=False, debug=False, num_devices=NCORES)

    xt_d = nc.dram_tensor("xt", [C, S], FP8, kind="ExternalInput").ap()
    xtb_d = nc.dram_tensor("xtb", [C, B * 512], BF16, kind="ExternalInput").ap()
    wqkv_d = nc.dram_tensor("wqkv", [C, 6 * D], FP8, kind="ExternalInput").ap()
    wqkvb_d = nc.dram_tensor("wqkvb", [C, 6 * D], BF16, kind="ExternalInput").ap()
    wout_d = nc.dram_tensor("wout", [HPC * D, C], BF16, kind="ExternalInput").ap()
    wout8_d = nc.dram_tensor("wout8", [HPC * D, C], FP8, kind="ExternalInput").ap()
    cos_d = nc.dram_tensor("cos2t", [D, T], F32, kind="ExternalInput").ap()
    sin_d = nc.dram_tensor("sin2t", [D, T], F32, kind="ExternalInput").ap()
    mask_d = nc.dram_tensor("maskadd", [128, 512], BF16, kind="ExternalInput").ap()
    ident_d = nc.dram_tensor("ident_in", [128, 128], BF16, kind="ExternalInput").ap()
    ones_d = nc.dram_tensor("ones_in", [128, 1], BF16, kind="ExternalInput").ap()
    y_d = nc.dram_tensor("y", [S, C], BF16, kind="ExternalOutput").ap()

    xt_t = xt_d.rearrange("(ct p) s -> p ct s", p=128)  # [128, 16, 4096]
    xtb_t = xtb_d.rearrange("(ct p) s -> p ct s", p=128)  # [128, 16, 1024]
    wqkv_t = wqkv_d.rearrange("(ct p) n -> p ct n", p=128)  # [128, 16, 768]
    wqkvb_t = wqkvb_d.rearrange("(ct p) n -> p ct n", p=128)  # [128, 16, 768]
    wout_t = wout_d.rearrange("(h p) n -> p h n", p=128)  # [128, 2, 2048]
    wout8_t = wout8_d.rearrange("(h p) n -> p h n", p=128)

    Exp = mybir.ActivationFunctionType.Exp
    Copy = mybir.ActivationFunctionType.Copy

    with tile.TileContext(nc) as tc:
        with (
            tc.tile_pool(name="cst", bufs=1) as cst,
            tc.tile_pool(name="sx", bufs=2) as sx,
            tc.tile_pool(name="sqk", bufs=1) as sqk,
            tc.tile_pool(name="srp", bufs=3) as srp,
            tc.tile_pool(name="se", bufs=6) as se,
            tc.tile_pool(name="snb", bufs=2) as snb,
            tc.tile_pool(name="sou", bufs=4) as sou,
            tc.tile_pool(name="sy", bufs=12) as sy,
            # PSUM: 8 banks total.
            tc.tile_pool(name="pqs", bufs=2, space="PSUM") as pqs,  # qkv ps + scores
            tc.tile_pool(name="pvc", bufs=2, space="PSUM") as pvc,  # v ps + colsum
            tc.tile_pool(name="pou", bufs=2, space="PSUM") as pou,  # out_un
            tc.tile_pool(name="py", bufs=2, space="PSUM") as py,  # proj out
        ):
            # ---- resident constants, ordered so the first qkv groups can
            # start ASAP: sync ring interleaves xt pieces with wqkv head
            # slices; scalar ring carries block-0 cos/sin then the rest.
            xt_first = sx.tile([128, NCT, 512], BF16, tag="xtb", name="xtb")
            wqkv = cst.tile([128, NCT, 6 * D], FP8, tag="wqkv", name="wqkv")
            wqkvb = cst.tile([128, NCT, 6 * D], BF16, tag="wqkvb", name="wqkvb")
            cos_sb = cst.tile([128, T], F32, tag="cos", name="cos_sb")
            sin_sb = cst.tile([128, T], F32, tag="sin", name="sin_sb")
            wout = cst.tile([128, HPC, C], BF16, tag="wout", name="wout")

            def _wq(ct, eng):
                eng.dma_start(wqkvb[:, ct, :], wqkvb_t[:, ct, :])

            def _xp(q):
                nc.sync.dma_start(
                    xt_first[:, ds(4 * q, 4), :], xtb_t[:, ds(4 * q, 4), ds(0, 512)]
                )

            for q in range(4):
                _xp(q)
                for ct in range(4 * q, 4 * q + 4):
                    if ct % 2 == 1:
                        _wq(ct, nc.sync)
            for ct in range(0, NCT, 2):
                _wq(ct, nc.scalar)
            nc.scalar.dma_start(cos_sb[:, ds(0, 512)], cos_d[:, ds(0, 512)])
            nc.scalar.dma_start(sin_sb[:, ds(0, 512)], sin_d[:, ds(0, 512)])
            nc.scalar.dma_start(wqkv[:], wqkv_t)  # fp8 weights for blocks 1-3
            nc.scalar.dma_start(cos_sb[:, ds(512, T - 512)], cos_d[:, ds(512, T - 512)])
            nc.scalar.dma_start(sin_sb[:, ds(512, T - 512)], sin_d[:, ds(512, T - 512)])
            nc.scalar.dma_start(wout[:], wout_t)
            wout8 = cst.tile([128, HPC, C], FP8, tag="wout8", name="wout8")
            nc.scalar.dma_start(wout8[:], wout8_t)
            mask = cst.tile([128, 512], BF16, tag="mask", name="mask")
            nc.gpsimd.dma_start(mask[:], mask_d)
            ident = cst.tile([128, 128], BF16, tag="ident", name="ident")
            nc.gpsimd.dma_start(ident[:], ident_d)
            ones = cst.tile([128, 1], BF16, tag="ones", name="ones")
            nc.gpsimd.dma_start(ones[:], ones_d)
            zeros = cst.tile([128, 512], BF16, tag="zeros", name="zeros")
            nc.gpsimd.memset(zeros[:], 0.0)

            def proj_unit(b, j, ou, tt, cb, k, alt=False):
                """One (tq tile, C block) unit of the output projection.

                ou is ('b', [ousb_h0, ousb_h1]) for the bf16 path (early
                tokens) or ('f', ou2) for the fp8 DoubleRow path, where ou2
                is [128(d), 2(head), 512(tq)] pre-scaled by WSCALE."""
                yps = py.tile([128, 512], F32, tag="y", name="yps")
                if ou[0] == "f":
                    nc.tensor.matmul(
                        yps[:],
                        ou[1][:, :, ds(tt * 128, 128)],
                        wout8[:, :, ds(cb * 512, 512)],
                        start=True,
                        stop=True,
                        perf_mode=mybir.MatmulPerfMode.DoubleRow,
                    )
                    unscale = 1.0 / (WSCALE * WSCALE)
                else:
                    for h in range(HPC):
                        nc.tensor.matmul(
                            yps[:],
                            ou[1][h][:, ds(tt * 128, 128)],
                            wout[:, h, ds(cb * 512, 512)],
                            start=(h == 0),
                            stop=(h == HPC - 1),
                        )
                    unscale = 1.0
                ysb = sy.tile([128, 512], BF16, tag="ysb", name="ysb")
                if (k % 2 == 1) if alt else (k % 3 == 2):  # ACT copy share
                    nc.scalar.activation(ysb[:], yps[:], Copy, scale=unscale)
                else:
                    # scalar_tensor_tensor, NOT tensor_copy/tensor_scalar:
                    # those enter DVE 2-port perf mode and lock the shared
                    # SBUF port against gpsimd (broadcast + SWDGE y-DMAs)
                    nc.vector.scalar_tensor_tensor(
                        ysb[:],
                        yps[:],
                        unscale,
                        zeros[:],
                        op0=mybir.AluOpType.mult,
                        op1=mybir.AluOpType.add,
                    )
                if alt:  # final block: both HWDGE rings (idle gpsimd drain)
                    yeng = nc.sync if k % 2 == 0 else nc.scalar
                else:
                    yeng = nc.sync if k % 2 == 0 else nc.gpsimd
                yeng.dma_start(
                    y_d[ds(b * T + j * 512 + tt * 128, 128), ds(cb * 512, 512)],
                    ysb[:],
                )

            UNITS = [(tt, cb) for tt in range(4) for cb in range(NBLK)]

            def proj_block(b, j, ou_sb, alt=False):
                """Project tq block j of batch b through w_out and DMA out."""
                for k, (tt, cb) in enumerate(UNITS):
                    proj_unit(b, j, ou_sb, tt, cb, k, alt=alt)

            pending = None  # (b, ou_sb) of the previous batch's last tq block

            for rep in range(reps):
             for b in range(B):
                # ================= qkv projection + RoPE =================
                qk = [
                    sqk.tile([128, T], BF16, tag=f"qk{i}", name=f"qk{i}")
                    for i in range(4)  # q0 q1 k0 k1
                ]
                v_sb = sqk.tile([128, NTK, HPC * D], BF16, tag="v", name="v_sb")

                for blk in range(NBLK):  # 4 token blocks of 512
                    tok0 = b * T + blk * 512
                    bf_blk = blk == 0  # first block per batch: bf16 (accuracy)
                    if rep == 0 and b == 0 and blk == 0:
                        xt = xt_first
                    elif bf_blk:
                        xt = sx.tile([128, NCT, 512], BF16, tag="xtb", name="xtb")
                        nc.sync.dma_start(xt[:], xtb_t[:, :, ds(b * 512, 512)])
                    else:
                        xt = sx.tile([128, NCT, 512], FP8, tag="xt", name="xt")
                        nc.sync.dma_start(xt[:], xt_t[:, :, ds(tok0, 512)])

                    if blk == 1 and pending is not None:
                        # previous batch's last tq block projects here, after
                        # the first qkv group has covered its norm latency
                        proj_block(pending[0], NBLK - 1, pending[1])
                        pending = None

                    for ht in range(4):  # q0 q1 k0 k1
                        ps = pqs.tile([128, 512], F32, tag="m", name="qk_ps")
                        if bf_blk:
                            for ct in range(NCT):
                                nc.tensor.matmul(
                                    ps[:],
                                    wqkvb[:, ct, ds(ht * D, D)],
                                    xt[:, ct, :],
                                    start=(ct == 0),
                                    stop=(ct == NCT - 1),
                                )
                        else:
                            for cp in range(NCT // 2):  # fp8 DoubleRow ct-pairs
                                nc.tensor.matmul(
                                    ps[:],
                                    wqkv[:, ds(2 * cp, 2), ds(ht * D, D)],
                                    xt[:, ds(2 * cp, 2), :],
                                    start=(cp == 0),
                                    stop=(cp == NCT // 2 - 1),
                                    perf_mode=mybir.MatmulPerfMode.DoubleRow,
                                )
                        # RoPE: qk_blk = ps*cos + swap_halves(ps)*sin_signed
                        cs = cos_sb[:, ds(blk * 512, 512)]
                        sn = sin_sb[:, ds(blk * 512, 512)]
                        shuf = srp.tile([128, 512], F32, tag="shuf", name="shuf")
                        nc.scalar.copy(shuf[0:64, :], ps[64:128, :])
                        nc.scalar.copy(shuf[64:128, :], ps[0:64, :])
                        nc.vector.tensor_mul(shuf[:], shuf[:], sn)
                        tmp = srp.tile([128, 512], F32, tag="tmp", name="tmp")
                        nc.vector.tensor_mul(tmp[:], ps[:], cs)
                        nc.vector.tensor_add(
                            qk[ht][:, ds(blk * 512, 512)], tmp[:], shuf[:]
                        )

                    for half in range(2):  # 2 v psum tiles per block
                        vps = pvc.tile([128, 2, HPC * D], F32, tag="vc", name="v_ps")
                        for cch in range(2):
                            chunk = half * 2 + cch  # 128-token chunk in blk
                            if bf_blk:
                                for ct in range(NCT):
                                    nc.tensor.matmul(
                                        vps[:, cch, :],
                                        xt[:, ct, ds(chunk * 128, 128)],
                                        wqkvb[:, ct, ds(4 * D, HPC * D)],
                                        start=(ct == 0),
                                        stop=(ct == NCT - 1),
                                    )
                            else:
                                for cp in range(NCT // 2):
                                    nc.tensor.matmul(
                                        vps[:, cch, :],
                                        xt[:, ds(2 * cp, 2), ds(chunk * 128, 128)],
                                        wqkv[:, ds(2 * cp, 2), ds(4 * D, HPC * D)],
                                        start=(cp == 0),
                                        stop=(cp == NCT // 2 - 1),
                                        perf_mode=mybir.MatmulPerfMode.DoubleRow,
                                    )
                        i0 = blk * 4 + half * 2
                        nc.scalar.activation(
                            v_sb[:, ds(i0, 2), :].rearrange("p a b -> p (a b)"),
                            vps[:].rearrange("p a b -> p (a b)"),
                            Copy,
                            scale=1.0 / WSCALE,
                        )

                # ================= attention (+ inlined projection) ======
                prev_ou = None
                for j in range(NBLK):
                    ou_sb = []
                    if j > 0:
                        ou2 = sou.tile(
                            [128, HPC, 512], FP8, tag="ou2", name="ou2", bufs=2
                        )
                    units_left = list(enumerate(UNITS)) if j > 0 else []
                    for h in range(HPC):
                        qT, kT = qk[h], qk[2 + h]
                        ntk = 4 * j + 4
                        ou_ps = pou.tile([128, 512], F32, tag="ou", name="ou_ps")
                        cs_ps = pvc.tile([1, 512], F32, tag="vc", name="cs_ps")

                        def scores(i):
                            sp = pqs.tile([128, 512], F32, tag="m", name="sp")
                            rr = i - 4 * j
                            nc.tensor.matmul(
                                sp[:],
                                kT[:, ds(i * 128, 128)],
                                qT[:, ds(j * 512, 512)],
                                start=True,
                                stop=(rr < 0),
                            )
                            if rr >= 0:
                                # causal mask added on PE: the slice of the
                                # composite mask constant covers the fully
                                # masked columns AND the triangular square
                                w = (rr + 1) * 128
                                nc.tensor.matmul(
                                    sp[:, ds(0, w)],
                                    ident[:],
                                    mask[:, ds((3 - rr) * 128, w)],
                                    start=False,
                                    stop=True,
                                )
                            return sp

                        def exp_of(i, sp):
                            # masked scores carry -1e30 from the PE mask add,
                            # so a single full-width exp yields exact zeros
                            e = se.tile([128, 512], BF16, tag="e", name="e")
                            nc.scalar.activation(e[:], sp[:], Exp, scale=EXP_SCALE)
                            return e

                        DEPTH = 2
                        nquad = ntk // 4
                        es = []
                        for i in range(min(DEPTH, ntk)):
                            es.append(exp_of(i, scores(i)))
                        for i in range(ntk):
                            if i + DEPTH < ntk:
                                es.append(exp_of(i + DEPTH, scores(i + DEPTH)))
                            e = es[i]
                            nc.tensor.matmul(
                                ou_ps[:],
                                v_sb[:, i, ds(h * D, D)],
                                e[:],
                                start=(i == 0),
                                stop=(i == ntk - 1),
                            )
                            # colsum: sum quads of e tiles on DVE (bf16, 2x
                            # rate) so PE streams 1 colsum matmul per 4 tiles.
                            if i % 4 == 1:
                                e2 = se.tile(
                                    [128, 512], BF16, tag="e2", name="e2", bufs=2
                                )
                                nc.vector.tensor_add(e2[:], es[i - 1][:], e[:])
                                last_e2 = e2
                            elif i % 4 == 3:
                                e4 = se.tile(
                                    [128, 512], BF16, tag="e4", name="e4", bufs=2
                                )
                                nc.vector.tensor_add(e4[:], es[i - 1][:], e[:])
                                nc.vector.tensor_add(e4[:], e4[:], last_e2[:])
                                iq = i // 4
                                nc.tensor.matmul(
                                    cs_ps[:],
                                    ones[:],
                                    e4[:],
                                    start=(iq == 0),
                                    stop=(iq == nquad - 1),
                                )
                            # backfill PE with prev block's projection while
                            # ACT exp is the rate limiter in this loop (first
                            # unit only after out(0), so the previous norm
                            # chain latency hides under the exp(0) wait)
                            npop = 2 if i < 4 else (1 if i % 2 == 1 else 0)
                            for _ in range(npop):
                                if units_left:
                                    k, (tt, cb) = units_left.pop(0)
                                    proj_unit(b, j - 1, prev_ou, tt, cb, k)

                        row = snb.tile([1, 512], F32, tag="row", name="row")
                        nc.vector.reciprocal_approx_fast(row[:], cs_ps[:])
                        bc = snb.tile([128, 512], F32, tag="bc", name="bc")
                        nc.gpsimd.partition_broadcast(bc[:], row[0:1, :])
                        if j > 0:
                            # fp8 out tile, pre-scaled by WSCALE for the
                            # DoubleRow projection
                            nc.vector.scalar_tensor_tensor(
                                ou2[:, h, :],
                                ou_ps[:],
                                WSCALE,
                                bc[:],
                                op0=mybir.AluOpType.mult,
                                op1=mybir.AluOpType.mult,
                            )
                        else:
                            ousb = sou.tile(
                                [128, 512], BF16, tag="ou", name="ousb"
                            )
                            nc.vector.tensor_mul(ousb[:], ou_ps[:], bc[:])
                            ou_sb.append(ousb)

                    for k, (tt, cb) in units_left:  # flush remaining units
                        proj_unit(b, j - 1, prev_ou, tt, cb, k)
                    prev_ou = ("f", ou2) if j > 0 else ("b", ou_sb)
                pending = (b, prev_ou)
             if rep == reps - 1:
                proj_block(pending[0], NBLK - 1, pending[1], alt=True)
             # (non-final reps hand their last block to the next rep's qkv)

    nc.compile()
    return nc


def _host_prep(x, w_qkv, w_out, cos, sin):
    x = np.asarray(x, dtype=np.float32)
    w_qkv = np.asarray(w_qkv, dtype=np.float32)
    w_out = np.asarray(w_out, dtype=np.float32)
    cos = np.asarray(cos, dtype=np.float32)
    sin = np.asarray(sin, dtype=np.float32)

    bf16 = ml_dtypes.bfloat16
    fp8 = mybir.dt.np(FP8)
    xt_f = np.ascontiguousarray(x.reshape(S, C).T)  # [C, S]
    xt = xt_f.astype(fp8)
    # bf16 copy of the first 512-token block of each batch (see kernel doc)
    xtb = np.concatenate(
        [xt_f[:, b * T : b * T + 512] for b in range(B)], axis=1
    ).astype(bf16)
    # cos/sin tables carry the 1/WSCALE that undoes the w_qkv pre-scale
    cos2t = np.ascontiguousarray(np.concatenate([cos, cos], axis=1).T) / WSCALE
    sin2t = np.ascontiguousarray(np.concatenate([-sin, sin], axis=1).T) / WSCALE
    # composite causal mask: cols 0-383 fully masked, cols 384-511 the
    # strictly-lower-triangle square; slice [(3-rr)*128 : 512] serves every
    # diagonal tile position rr
    tri = np.tril(np.full((128, 128), NEG, dtype=np.float32), k=-1)
    maskadd = np.concatenate(
        [np.full((128, 384), NEG, dtype=np.float32), tri], axis=1
    ).astype(bf16)
    ident = np.eye(128, dtype=bf16)
    ones = np.ones((128, 1), dtype=bf16)

    in_maps = []
    for c in range(NCORES):
        h0 = c * HPC
        cols = []
        for qkv_i in range(3):
            for h in range(HPC):
                base = qkv_i * C + (h0 + h) * D
                cols.append(w_qkv[:, base : base + D])
        wqkv_f = np.concatenate(cols, axis=1) * WSCALE  # [C, 768]
        wqkv_c = wqkv_f.astype(fp8)
        wqkvb_c = wqkv_f.astype(bf16)
        wout_f = w_out[h0 * D : (h0 + HPC) * D, :]  # [256, C]
        wout_c = wout_f.astype(bf16)
        wout8_c = (wout_f * WSCALE).astype(fp8)
        in_maps.append(
            {
                "xt": xt,
                "xtb": xtb,
                "wqkvb": np.ascontiguousarray(wqkvb_c),
                "wqkv": np.ascontiguousarray(wqkv_c),
                "wout": np.ascontiguousarray(wout_c),
                "wout8": np.ascontiguousarray(wout8_c),
                "cos2t": cos2t,
                "sin2t": sin2t,
                "maskadd": maskadd,
                "ident_in": ident,
                "ones_in": ones,
            }
        )
    return in_maps


def _get_runner(reps=1):
    """Build (once) a jitted shard_map callable running the NEFF on 8 cores."""
    key = ("runner", reps)
    if key in _CACHE:
        return _CACHE[key]

    import jax
    from jax.sharding import Mesh, PartitionSpec
    try:
        from jax.experimental.shard_map import shard_map
    except ImportError:  # newer jax
        from jax.shard_map import shard_map  # type: ignore
    from concourse import bass2jax

    nckey = ("nc", reps)
    nc = _CACHE.get(nckey)
    if nc is None:
        nc = _CACHE[nckey] = build_nc(reps)
    bass2jax.install_neuronx_cc_hook()

    partition_name = (
        nc.partition_id_tensor.name if nc.partition_id_tensor else None
    )
    in_names, out_names, out_avals = [], [], []
    for alloc in nc.m.functions[0].allocations:
        if not isinstance(alloc, mybir.MemoryLocationSet):
            continue
        name = alloc.memorylocations[0].name
        if alloc.kind == "ExternalInput":
            if name != partition_name:
                in_names.append(name)
        elif alloc.kind == "ExternalOutput":
            out_names.append(name)
            out_avals.append(
                jax.core.ShapedArray(
                    tuple(alloc.tensor_shape), mybir.dt.np(alloc.dtype)
                )
            )
    n_params = len(in_names)
    all_names = in_names + out_names
    if partition_name is not None:
        all_names = all_names + [partition_name]

    def _body(*args):
        operands = list(args)
        if partition_name is not None:
            operands.append(bass2jax.partition_id_tensor())
        outs = bass2jax._bass_exec_p.bind(
            *operands,
            out_avals=tuple(out_avals),
            in_names=tuple(all_names),
            out_names=tuple(out_names),
            lowering_input_output_aliases=(),
            sim_require_finite=True,
            sim_require_nnan=True,
            nc=nc,
        )
        return tuple(outs)

    devices = jax.devices()[:NCORES]
    mesh = Mesh(np.asarray(devices), ("core",))
    nin = n_params + len(out_names)
    sharded = jax.jit(
        shard_map(
            _body,
            mesh=mesh,
            in_specs=(PartitionSpec("core"),) * nin,
            out_specs=(PartitionSpec("core"),) * len(out_names),
            check_rep=False,
        ),
        keep_unused=True,
    )
    zeros = [
        np.zeros((NCORES * a.shape[0], *a.shape[1:]), a.dtype) for a in out_avals
    ]
    _CACHE[key] = (sharded, in_names, out_names, out_avals, zeros, mesh)
    return _CACHE[key]


def _concat_inputs(in_maps, in_names):
    return [
        np.concatenate([m[nm] for m in in_maps], axis=0) for nm in in_names
    ]


def _run(in_maps):
    sharded, in_names, out_names, out_avals, zeros, mesh = _get_runner()
    concat_in = _concat_inputs(in_maps, in_names)
    out = sharded(*concat_in, *zeros)
    y = np.asarray(out[out_names.index("y")])
    return y.reshape(NCORES, S, C)


def kernel(x, w_qkv, w_out, cos, sin):
    in_maps = _host_prep(x, w_qkv, w_out, cos, sin)
    parts = _run(in_maps)
    acc = parts.astype(np.float32).sum(axis=0)
    return acc.reshape(B, T, C)


def time_exec(x, w_qkv, w_out, cos, sin, iters=10, reps=1):
    """Time device execution with device-resident inputs (excludes upload)."""
    import time as _time
    import jax

    sharded, in_names, out_names, out_avals, zeros, mesh = _get_runner(reps)
    in_maps = _host_prep(x, w_qkv, w_out, cos, sin)
    args = [jax.device_put(a) for a in _concat_inputs(in_maps, in_names)]
    zs = [jax.device_put(z) for z in zeros]
    out = sharded(*args, *zs)  # warm-up + compile
    jax.block_until_ready(out)
    times = []
    for _ in range(iters):
        t0 = _time.perf_counter()
        out = sharded(*args, *zs)
        jax.block_until_ready(out)
        times.append(_time.perf_counter() - t0)
    return times


# revision 68
# speedup vs baseline: 1.0691x; 1.0691x over previous
"""Trainium2 Bass kernel for causal multi-head attention with RoPE.

Model: B=2, T=2048, C=2048, H=16 heads, D=128 head_dim.
  qkv = x @ w_qkv ; q,k rotary-embedded ; causal softmax attention ; out @ w_out.

Sharding: tensor-parallel over heads. 16 heads / 8 cores = 2 heads per core.
Each core gets w_qkv columns and w_out rows for its 2 heads, computes a full
(B*T, C) partial output projection (bf16), and the host sums the 8 partials.

Mixed precision, chosen so the max-relative-error metric (dominated by early
tokens, whose concentrated softmax makes outputs large and error-proportional)
stays well under budget while late tokens ride cheap fp8:
  - token block 0 of each batch: bf16 x / w_qkv projections.
  - token blocks 1-3: fp8-e4m3 x / w_qkv with DoubleRow matmuls (256-deep
    contraction, half the PE instructions).  w_qkv is pre-scaled by WSCALE to
    clear fp8's denormal range; the scale is undone for free in the cos/sin
    tables (q,k) and the v-copy activation scale.
  - attention (scores/exp/out) in bf16; fp32 PSUM accumulation everywhere.
  - output projection: tq block 0 in bf16; blocks 1-3 contract both local
    heads in one fp8 DoubleRow matmul against WSCALE-scaled w_out, unscaled
    during the PSUM->SBUF copy.

Per-core dataflow:
  - x is fed pre-transposed (xT, [C, B*T]) so the C contraction sits on
    partitions.  qT/kT come out of the projection directly in [D, T] layout
    (D on partitions), v in natural [T, D] layout.
  - RoPE on DVE in [D, T] layout: rot_half is a partition-half swap done with
    two ACT copies, the sign folded into the sin table host-side.
  - scores computed transposed (sT[tk, tq] = kT.T @ qT); the causal mask
    (fully-masked columns AND the diagonal triangle, one composite constant
    sliced per diagonal position) is added on PE via an identity x mask
    matmul into the same PSUM bank, so every exp is one uniform full-width
    ACT op (masked entries underflow to zero) with 1/sqrt(D) folded in.
  - out_un[d, tq] accumulates v.T @ expT on PE; softmax denominators via a
    ones-column matmul over DVE-summed quads of e tiles.
  - normalization: fast approximate reciprocal of the colsum row on DVE,
    gpsimd partition-broadcast, one DVE multiply.
  - projection units are interleaved into the attention loops (PE backfill
    while ACT exp is the rate limiter) and stream to HBM as bf16 partials.
"""

import numpy as np
import ml_dtypes

import concourse.bass as bass
import concourse.tile as tile
import concourse.mybir as mybir
from concourse import bacc
from concourse.bass import ds
from concourse.bass_utils import run_bass_kernel_spmd

B, T, C, H, D = 2, 2048, 2048, 16, 128
NCORES = 8
HPC = H // NCORES  # heads per core = 2
S = B * T  # 4096 tokens
NBLK = T // 512  # 4 tq blocks of 512 per batch
NCT = C // 128  # 16 contraction tiles for the qkv projection
NTK = T // 128  # 16 tk tiles per batch
F32 = mybir.dt.float32
BF16 = mybir.dt.bfloat16
FP8 = mybir.dt.float8e4
WSCALE = 64.0  # w_qkv pre-scaled by this (fp8 denormal avoidance); undone in
               # the cos/sin tables (q,k) and the v-copy activation scale
EXP_SCALE = float(D) ** -0.5
NEG = -1.0e30

_CACHE = {}


def build_nc(reps=1):
    nc = bacc.Bacc("TRN2", target_bir_lowering=False, debug=False, num_devices=NCORES)

    xt_d = nc.dram_tensor("xt", [C, S], FP8, kind="ExternalInput").ap()
    xtb_d = nc.dram_tensor("xtb", [C, B * 512], BF16, kind="ExternalInput").ap()
    wqkv_d = nc.dram_tensor("wqkv", [C, 6 * D], FP8, kind="ExternalInput").ap()
    wqkvb_d = nc.dram_tensor("wqkvb", [C, 6 * D], BF16, kind="ExternalInput").ap()
    wout_d = nc.dram_tensor("wout", [HPC * D, C], BF16, kind="ExternalInput").ap()
    wout8_d = nc.dram_tensor("wout8", [HPC * D, C], FP8, kind="ExternalInput").ap()
    cos_d = nc.dram_tensor("cos2t", [D, T], F32, kind="ExternalInput").ap()
    sin_d = nc.dram_tensor("sin2t", [D, T], F32, kind="ExternalInput").ap()
    mask_d = nc.dram_tensor("maskadd", [128, 512], BF16, kind="ExternalInput").ap()
    ident_d = nc.dram_tensor("ident_in", [128, 128], BF16, kind="ExternalInput").ap()
    ones_d = nc.dram_tensor("ones_in", [128, 1], BF16, kind="ExternalInput").ap()
    y_d = nc.dram_tensor("y", [S, C], BF16, kind="ExternalOutput").ap()

    xt_t = xt_d.rearrange("(ct p) s -> p ct s", p=128)  # [128, 16, 4096]
    xtb_t = xtb_d.rearrange("(ct p) s -> p ct s", p=128)  # [128, 16, 1024]
    wqkv_t = wqkv_d.rearrange("(ct p) n -> p ct n", p=128)  # [128, 16, 768]
    wqkvb_t = wqkvb_d.rearrange("(ct p) n -> p ct n", p=128)  # [128, 16, 768]
    wout_t = wout_d.rearrange("(h p) n -> p h n", p=128)  # [128, 2, 2048]
    wout8_t = wout8_d.rearrange("(h p) n -> p h n", p=128)

    Exp = mybir.ActivationFunctionType.Exp
    Copy = mybir.ActivationFunctionType.Copy

    with tile.TileContext(nc) as tc:
        with (
            tc.tile_pool(name="cst", bufs=1) as cst,
            tc.tile_pool(name="sx", bufs=2) as sx,
            tc.tile_pool(name="sqk", bufs=1) as sqk,
            tc.tile_pool(name="srp", bufs=3) as srp,
            tc.tile_pool(name="se", bufs=6) as se,
            tc.tile_pool(name="snb", bufs=2) as snb,
            tc.tile_pool(name="sou", bufs=4) as sou,
            tc.tile_pool(name="sy", bufs=12) as sy,
            # PSUM: 8 banks total.
            tc.tile_pool(name="pqs", bufs=3, space="PSUM") as pqs,  # qkv ps + scores
            tc.tile_pool(name="pvc", bufs=1, space="PSUM") as pvc,  # v ps + colsum
            tc.tile_pool(name="pou", bufs=2, space="PSUM") as pou,  # out_un
            tc.tile_pool(name="py", bufs=2, space="PSUM") as py,  # proj out
        ):
            # ---- resident constants, ordered so the first qkv groups can
            # start ASAP: sync ring interleaves xt pieces with wqkv head
            # slices; scalar ring carries block-0 cos/sin then the rest.
            xt_first = sx.tile([128, NCT, 512], BF16, tag="xtb", name="xtb")
            wqkv = cst.tile([128, NCT, 6 * D], FP8, tag="wqkv", name="wqkv")
            wqkvb = cst.tile([128, NCT, 6 * D], BF16, tag="wqkvb", name="wqkvb")
            cos_sb = cst.tile([128, T], F32, tag="cos", name="cos_sb")
            sin_sb = cst.tile([128, T], F32, tag="sin", name="sin_sb")
            wout = cst.tile([128, HPC, C], BF16, tag="wout", name="wout")

            def _wq(ct, eng):
                eng.dma_start(wqkvb[:, ct, :], wqkvb_t[:, ct, :])

            def _xp(q, eng):
                eng.dma_start(
                    xt_first[:, ds(4 * q, 4), :], xtb_t[:, ds(4 * q, 4), ds(0, 512)]
                )

            _xp(0, nc.sync)
            for ct in range(1, NCT, 2):
                if ct == 9:
                    _xp(1, nc.sync)
                _wq(ct, nc.sync)
            for ct in range(0, NCT, 2):
                if ct == 8:
                    _xp(2, nc.scalar)
                elif ct == 12:
                    _xp(3, nc.scalar)
                _wq(ct, nc.scalar)
            nc.scalar.dma_start(cos_sb[:, ds(0, 512)], cos_d[:, ds(0, 512)])
            nc.scalar.dma_start(sin_sb[:, ds(0, 512)], sin_d[:, ds(0, 512)])
            nc.scalar.dma_start(wqkv[:], wqkv_t)  # fp8 weights for blocks 1-3
            nc.scalar.dma_start(cos_sb[:, ds(512, T - 512)], cos_d[:, ds(512, T - 512)])
            nc.scalar.dma_start(sin_sb[:, ds(512, T - 512)], sin_d[:, ds(512, T - 512)])
            nc.scalar.dma_start(wout[:], wout_t)
            wout8 = cst.tile([128, HPC, C], FP8, tag="wout8", name="wout8")
            nc.scalar.dma_start(wout8[:], wout8_t)
            mask = cst.tile([128, 512], BF16, tag="mask", name="mask")
            nc.gpsimd.dma_start(mask[:], mask_d)
            ident = cst.tile([128, 128], BF16, tag="ident", name="ident")
            nc.gpsimd.dma_start(ident[:], ident_d)
            ones = cst.tile([128, 1], BF16, tag="ones", name="ones")
            nc.gpsimd.dma_start(ones[:], ones_d)
            zeros = cst.tile([128, 512], BF16, tag="zeros", name="zeros")
            nc.gpsimd.memset(zeros[:], 0.0)

            def proj_unit(b, j, ou, tt, cb, k, alt=False):
                """One (tq tile, C block) unit of the output projection.

                ou is ('b', [ousb_h0, ousb_h1]) for the bf16 path (early
                tokens) or ('f', ou2) for the fp8 DoubleRow path, where ou2
                is [128(d), 2(head), 512(tq)] pre-scaled by WSCALE."""
                yps = py.tile([128, 512], F32, tag="y", name="yps")
                if ou[0] == "f":
                    nc.tensor.matmul(
                        yps[:],
                        ou[1][:, :, ds(tt * 128, 128)],
                        wout8[:, :, ds(cb * 512, 512)],
                        start=True,
                        stop=True,
                        perf_mode=mybir.MatmulPerfMode.DoubleRow,
                    )
                    unscale = 1.0 / (WSCALE * WSCALE)
                else:
                    for h in range(HPC):
                        nc.tensor.matmul(
                            yps[:],
                            ou[1][h][:, ds(tt * 128, 128)],
                            wout[:, h, ds(cb * 512, 512)],
                            start=(h == 0),
                            stop=(h == HPC - 1),
                        )
                    unscale = 1.0
                ysb = sy.tile([128, 512], BF16, tag="ysb", name="ysb")
                if (k % 2 == 1) if alt else (k % 3 == 2):  # ACT copy share
                    nc.scalar.activation(ysb[:], yps[:], Copy, scale=unscale)
                else:
                    # scalar_tensor_tensor, NOT tensor_copy/tensor_scalar:
                    # those enter DVE 2-port perf mode and lock the shared
                    # SBUF port against gpsimd (broadcast + SWDGE y-DMAs)
                    nc.vector.scalar_tensor_tensor(
                        ysb[:],
                        yps[:],
                        unscale,
                        zeros[:],
                        op0=mybir.AluOpType.mult,
                        op1=mybir.AluOpType.add,
                    )
                if alt:  # final block: both HWDGE rings (idle gpsimd drain)
                    yeng = nc.sync if k % 2 == 0 else nc.scalar
                else:
                    yeng = nc.sync if k % 2 == 0 else nc.gpsimd
                yeng.dma_start(
                    y_d[ds(b * T + j * 512 + tt * 128, 128), ds(cb * 512, 512)],
                    ysb[:],
                )

            UNITS = [(tt, cb) for tt in range(4) for cb in range(NBLK)]

            def proj_block(b, j, ou_sb, alt=False):
                """Project tq block j of batch b through w_out and DMA out."""
                for k, (tt, cb) in enumerate(UNITS):
                    proj_unit(b, j, ou_sb, tt, cb, k, alt=alt)

            pending = None  # (b, ou_sb) of the previous batch's last tq block

            for rep in range(reps):
             for b in range(B):
                # ================= qkv projection + RoPE =================
                qk = [
                    sqk.tile([128, T], BF16, tag=f"qk{i}", name=f"qk{i}")
                    for i in range(4)  # q0 q1 k0 k1
                ]
                v_sb = sqk.tile([128, NTK, HPC * D], BF16, tag="v", name="v_sb")

                for blk in range(NBLK):  # 4 token blocks of 512
                    tok0 = b * T + blk * 512
                    bf_blk = blk == 0  # first block per batch: bf16 (accuracy)
                    if rep == 0 and b == 0 and blk == 0:
                        xt = xt_first
                    elif bf_blk:
                        xt = sx.tile([128, NCT, 512], BF16, tag="xtb", name="xtb")
                        nc.sync.dma_start(xt[:], xtb_t[:, :, ds(b * 512, 512)])
                    else:
                        xt = sx.tile([128, NCT, 512], FP8, tag="xt", name="xt")
                        nc.sync.dma_start(xt[:], xt_t[:, :, ds(tok0, 512)])

                    if blk == 1 and pending is not None:
                        # previous batch's last tq block projects here, after
                        # the first qkv group has covered its norm latency
                        proj_block(pending[0], NBLK - 1, pending[1])
                        pending = None

                    for ht in range(4):  # q0 q1 k0 k1
                        ps = pqs.tile([128, 512], F32, tag="m", name="qk_ps")
                        if bf_blk:
                            for ct in range(NCT):
                                nc.tensor.matmul(
                                    ps[:],
                                    wqkvb[:, ct, ds(ht * D, D)],
                                    xt[:, ct, :],
                                    start=(ct == 0),
                                    stop=(ct == NCT - 1),
                                )
                        else:
                            for cp in range(NCT // 2):  # fp8 DoubleRow ct-pairs
                                nc.tensor.matmul(
                                    ps[:],
                                    wqkv[:, ds(2 * cp, 2), ds(ht * D, D)],
                                    xt[:, ds(2 * cp, 2), :],
                                    start=(cp == 0),
                                    stop=(cp == NCT // 2 - 1),
                                    perf_mode=mybir.MatmulPerfMode.DoubleRow,
                                )
                        # RoPE: qk_blk = ps*cos + swap_halves(ps)*sin_signed
                        cs = cos_sb[:, ds(blk * 512, 512)]
                        sn = sin_sb[:, ds(blk * 512, 512)]
                        shuf = srp.tile([128, 512], F32, tag="shuf", name="shuf")
                        nc.scalar.copy(shuf[0:64, :], ps[64:128, :])
                        nc.scalar.copy(shuf[64:128, :], ps[0:64, :])
                        nc.vector.tensor_mul(shuf[:], shuf[:], sn)
                        tmp = srp.tile([128, 512], F32, tag="tmp", name="tmp")
                        nc.vector.tensor_mul(tmp[:], ps[:], cs)
                        nc.vector.tensor_add(
                            qk[ht][:, ds(blk * 512, 512)], tmp[:], shuf[:]
                        )

                    for half in range(2):  # 2 v psum tiles per block
                        vps = pvc.tile([128, 2, HPC * D], F32, tag="vc", name="v_ps")
                        for cch in range(2):
                            chunk = half * 2 + cch  # 128-token chunk in blk
                            if bf_blk:
                                for ct in range(NCT):
                                    nc.tensor.matmul(
                                        vps[:, cch, :],
                                        xt[:, ct, ds(chunk * 128, 128)],
                                        wqkvb[:, ct, ds(4 * D, HPC * D)],
                                        start=(ct == 0),
                                        stop=(ct == NCT - 1),
                                    )
                            else:
                                for cp in range(NCT // 2):
                                    nc.tensor.matmul(
                                        vps[:, cch, :],
                                        xt[:, ds(2 * cp, 2), ds(chunk * 128, 128)],
                                        wqkv[:, ds(2 * cp, 2), ds(4 * D, HPC * D)],
                                        start=(cp == 0),
                                        stop=(cp == NCT // 2 - 1),
                                        perf_mode=mybir.MatmulPerfMode.DoubleRow,
                                    )
                        i0 = blk * 4 + half * 2
                        nc.scalar.activation(
                            v_sb[:, ds(i0, 2), :].rearrange("p a b -> p (a b)"),
                            vps[:].rearrange("p a b -> p (a b)"),
                            Copy,
                            scale=1.0 / WSCALE,
                        )

                # ================= attention (+ inlined projection) ======
                prev_ou = None
                for j in range(NBLK):
                    ou_sb = []
                    if j > 0:
                        ou2 = sou.tile(
                            [128, HPC, 512], FP8, tag="ou2", name="ou2", bufs=2
                        )
                    units_left = list(enumerate(UNITS)) if j > 0 else []
                    for h in range(HPC):
                        qT, kT = qk[h], qk[2 + h]
                        ntk = 4 * j + 4
                        ou_ps = pou.tile([128, 512], F32, tag="ou", name="ou_ps")
                        cs_ps = pvc.tile([1, 512], F32, tag="vc", name="cs_ps")

                        def scores(i):
                            sp = pqs.tile([128, 512], F32, tag="m", name="sp")
                            rr = i - 4 * j
                            nc.tensor.matmul(
                                sp[:],
                                kT[:, ds(i * 128, 128)],
                                qT[:, ds(j * 512, 512)],
                                start=True,
                                stop=(rr < 0),
                            )
                            if rr >= 0:
                                # causal mask added on PE: the slice of the
                                # composite mask constant covers the fully
                                # masked columns AND the triangular square
                                w = (rr + 1) * 128
                                nc.tensor.matmul(
                                    sp[:, ds(0, w)],
                                    ident[:],
                                    mask[:, ds((3 - rr) * 128, w)],
                                    start=False,
                                    stop=True,
                                )
                            return sp

                        def exp_of(i, sp):
                            # masked scores carry -1e30 from the PE mask add,
                            # so a single full-width exp yields exact zeros
                            e = se.tile([128, 512], BF16, tag="e", name="e")
                            nc.scalar.activation(e[:], sp[:], Exp, scale=EXP_SCALE)
                            return e

                        DEPTH = 3
                        nquad = ntk // 4
                        es = []
                        for i in range(min(DEPTH, ntk)):
                            es.append(exp_of(i, scores(i)))
                        for i in range(ntk):
                            if i + DEPTH < ntk:
                                es.append(exp_of(i + DEPTH, scores(i + DEPTH)))
                            e = es[i]
                            nc.tensor.matmul(
                                ou_ps[:],
                                v_sb[:, i, ds(h * D, D)],
                                e[:],
                                start=(i == 0),
                                stop=(i == ntk - 1),
                            )
                            # colsum: sum quads of e tiles on DVE (bf16, 2x
                            # rate) so PE streams 1 colsum matmul per 4 tiles.
                            if i % 4 == 1:
                                e2 = se.tile(
                                    [128, 512], BF16, tag="e2", name="e2", bufs=2
                                )
                                nc.vector.tensor_add(e2[:], es[i - 1][:], e[:])
                                last_e2 = e2
                            elif i % 4 == 3:
                                e4 = se.tile(
                                    [128, 512], BF16, tag="e4", name="e4", bufs=2
                                )
                                nc.vector.tensor_add(e4[:], es[i - 1][:], e[:])
                                nc.vector.tensor_add(e4[:], e4[:], last_e2[:])
                                iq = i // 4
                                nc.tensor.matmul(
                                    cs_ps[:],
                                    ones[:],
                                    e4[:],
                                    start=(iq == 0),
                                    stop=(iq == nquad - 1),
                                )
                            # backfill PE with prev block's projection while
                            # ACT exp is the rate limiter in this loop (first
                            # unit only after out(0), so the previous norm
                            # chain latency hides under the exp(0) wait)
                            npop = 2 if i < 4 else (1 if i % 2 == 1 else 0)
                            for _ in range(npop):
                                if units_left:
                                    k, (tt, cb) = units_left.pop(0)
                                    proj_unit(b, j - 1, prev_ou, tt, cb, k)

                        row = snb.tile([1, 512], F32, tag="row", name="row")
                        nc.vector.reciprocal_approx_fast(row[:], cs_ps[:])
                        bc = snb.tile([128, 512], F32, tag="bc", name="bc")
                        nc.gpsimd.partition_broadcast(bc[:], row[0:1, :])
                        if j > 0:
                            # fp8 out tile, pre-scaled by WSCALE for the
                            # DoubleRow projection
                            nc.vector.scalar_tensor_tensor(
                                ou2[:, h, :],
                                ou_ps[:],
                                WSCALE,
                                bc[:],
                                op0=mybir.AluOpType.mult,
                                op1=mybir.AluOpType.mult,
                            )
                        else:
                            ousb = sou.tile(
                                [128, 512], BF16, tag="ou", name="ousb"
                            )
                            nc.vector.tensor_mul(ousb[:], ou_ps[:], bc[:])
                            ou_sb.append(ousb)

                    for k, (tt, cb) in units_left:  # flush remaining units
                        proj_unit(b, j - 1, prev_ou, tt, cb, k)
                    prev_ou = ("f", ou2) if j > 0 else ("b", ou_sb)
                pending = (b, prev_ou)
             if rep == reps - 1:
                proj_block(pending[0], NBLK - 1, pending[1], alt=True)
             # (non-final reps hand their last block to the next rep's qkv)

    nc.compile()
    return nc


def _host_prep(x, w_qkv, w_out, cos, sin):
    x = np.asarray(x, dtype=np.float32)
    w_qkv = np.asarray(w_qkv, dtype=np.float32)
    w_out = np.asarray(w_out, dtype=np.float32)
    cos = np.asarray(cos, dtype=np.float32)
    sin = np.asarray(sin, dtype=np.float32)

    bf16 = ml_dtypes.bfloat16
    fp8 = mybir.dt.np(FP8)
    xt_f = np.ascontiguousarray(x.reshape(S, C).T)  # [C, S]
    xt = xt_f.astype(fp8)
    # bf16 copy of the first 512-token block of each batch (see kernel doc)
    xtb = np.concatenate(
        [xt_f[:, b * T : b * T + 512] for b in range(B)], axis=1
    ).astype(bf16)
    # cos/sin tables carry the 1/WSCALE that undoes the w_qkv pre-scale
    cos2t = np.ascontiguousarray(np.concatenate([cos, cos], axis=1).T) / WSCALE
    sin2t = np.ascontiguousarray(np.concatenate([-sin, sin], axis=1).T) / WSCALE
    # composite causal mask: cols 0-383 fully masked, cols 384-511 the
    # strictly-lower-triangle square; slice [(3-rr)*128 : 512] serves every
    # diagonal tile position rr
    tri = np.tril(np.full((128, 128), NEG, dtype=np.float32), k=-1)
    maskadd = np.concatenate(
        [np.full((128, 384), NEG, dtype=np.float32), tri], axis=1
    ).astype(bf16)
    ident = np.eye(128, dtype=bf16)
    ones = np.ones((128, 1), dtype=bf16)

    in_maps = []
    for c in range(NCORES):
        h0 = c * HPC
        cols = []
        for qkv_i in range(3):
            for h in range(HPC):
                base = qkv_i * C + (h0 + h) * D
                cols.append(w_qkv[:, base : base + D])
        wqkv_f = np.concatenate(cols, axis=1) * WSCALE  # [C, 768]
        wqkv_c = wqkv_f.astype(fp8)
        wqkvb_c = wqkv_f.astype(bf16)
        wout_f = w_out[h0 * D : (h0 + HPC) * D, :]  # [256, C]
        wout_c = wout_f.astype(bf16)
        wout8_c = (wout_f * WSCALE).astype(fp8)
        in_maps.append(
            {
                "xt": xt,
                "xtb": xtb,
                "wqkvb": np.ascontiguousarray(wqkvb_c),
                "wqkv": np.ascontiguousarray(wqkv_c),
                "wout": np.ascontiguousarray(wout_c),
                "wout8": np.ascontiguousarray(wout8_c),
                "cos2t": cos2t,
                "sin2t": sin2t,
                "maskadd": maskadd,
                "ident_in": ident,
                "ones_in": ones,
            }
        )
    return in_maps


def _get_runner(reps=1):
    """Build (once) a jitted shard_map callable running the NEFF on 8 cores."""
    key = ("runner", reps)
    if key in _CACHE:
        return _CACHE[key]

    import jax
    from jax.sharding import Mesh, PartitionSpec
    try:
        from jax.experimental.shard_map import shard_map
    except ImportError:  # newer jax
        from jax.shard_map import shard_map  # type: ignore
    from concourse import bass2jax

    nckey = ("nc", reps)
    nc = _CACHE.get(nckey)
    if nc is None:
        nc = _CACHE[nckey] = build_nc(reps)
    bass2jax.install_neuronx_cc_hook()

    partition_name = (
        nc.partition_id_tensor.name if nc.partition_id_tensor else None
    )
    in_names, out_names, out_avals = [], [], []
    for alloc in nc.m.functions[0].allocations:
        if not isinstance(alloc, mybir.MemoryLocationSet):
            continue
        name = alloc.memorylocations[0].name
        if alloc.kind == "ExternalInput":
            if name != partition_name:
                in_names.append(name)
        elif alloc.kind == "ExternalOutput":
            out_names.append(name)
            out_avals.append(
                jax.core.ShapedArray(
                    tuple(alloc.tensor_shape), mybir.dt.np(alloc.dtype)
                )
            )
    n_params = len(in_names)
    all_names = in_names + out_names
    if partition_name is not None:
        all_names = all_names + [partition_name]

    def _body(*args):
        operands = list(args)
        if partition_name is not None:
            operands.append(bass2jax.partition_id_tensor())
        outs = bass2jax._bass_exec_p.bind(
            *operands,
            out_avals=tuple(out_avals),
            in_names=tuple(all_names),
            out_names=tuple(out_names),
            lowering_input_output_aliases=(),
            sim_require_finite=True,
            sim_require_nnan=True,
            nc=nc,
        )
        return tuple(outs)

    devices = jax.devices()[:NCORES]
    mesh = Mesh(np.asarray(devices), ("core",))
    nin = n_params + len(out_names)
    sharded = jax.jit(
        shard_map(
            _body,
            mesh=mesh,
            in_specs=(PartitionSpec("core"),) * nin,
            out_specs=(PartitionSpec("core"),) * len(out_names),
            check_rep=False,
        ),
        keep_unused=True,
    )
    zeros = [
        np.zeros((NCORES * a.shape[0], *a.shape[1:]), a.dtype) for a in out_avals
    ]
    _CACHE[key] = (sharded, in_names, out_names, out_avals, zeros, mesh)
    return _CACHE[key]


def _concat_inputs(in_maps, in_names):
    return [
        np.concatenate([m[nm] for m in in_maps], axis=0) for nm in in_names
    ]


def _run(in_maps):
    sharded, in_names, out_names, out_avals, zeros, mesh = _get_runner()
    concat_in = _concat_inputs(in_maps, in_names)
    out = sharded(*concat_in, *zeros)
    y = np.asarray(out[out_names.index("y")])
    return y.reshape(NCORES, S, C)


def kernel(x, w_qkv, w_out, cos, sin):
    in_maps = _host_prep(x, w_qkv, w_out, cos, sin)
    parts = _run(in_maps)
    acc = parts.astype(np.float32).sum(axis=0)
    return acc.reshape(B, T, C)


def time_exec(x, w_qkv, w_out, cos, sin, iters=10, reps=1):
    """Time device execution with device-resident inputs (excludes upload)."""
    import time as _time
    import jax

    sharded, in_names, out_names, out_avals, zeros, mesh = _get_runner(reps)
    in_maps = _host_prep(x, w_qkv, w_out, cos, sin)
    args = [jax.device_put(a) for a in _concat_inputs(in_maps, in_names)]
    zs = [jax.device_put(z) for z in zeros]
    out = sharded(*args, *zs)  # warm-up + compile
    jax.block_until_ready(out)
    times = []
    for _ in range(iters):
        t0 = _time.perf_counter()
        out = sharded(*args, *zs)
        jax.block_until_ready(out)
        times.append(_time.perf_counter() - t0)
    return times


# revision 80
# speedup vs baseline: 1.0750x; 1.0055x over previous
"""Trainium2 Bass kernel for causal multi-head attention with RoPE.

Model: B=2, T=2048, C=2048, H=16 heads, D=128 head_dim.
  qkv = x @ w_qkv ; q,k rotary-embedded ; causal softmax attention ; out @ w_out.

Sharding: tensor-parallel over heads. 16 heads / 8 cores = 2 heads per core.
Each core gets w_qkv columns and w_out rows for its 2 heads, computes a full
(B*T, C) partial output projection (bf16), and the host sums the 8 partials.

Mixed precision, chosen so the max-relative-error metric (dominated by early
tokens, whose concentrated softmax makes outputs large and error-proportional)
stays well under budget while late tokens ride cheap fp8:
  - token block 0 of each batch: bf16 x / w_qkv projections.
  - token blocks 1-3: fp8-e4m3 x / w_qkv with DoubleRow matmuls (256-deep
    contraction, half the PE instructions).  w_qkv is pre-scaled by WSCALE to
    clear fp8's denormal range; the scale is undone for free in the cos/sin
    tables (q,k) and the v-copy activation scale.
  - attention (scores/exp/out) in bf16; fp32 PSUM accumulation everywhere.
  - output projection: tq block 0 in bf16; blocks 1-3 contract both local
    heads in one fp8 DoubleRow matmul against WSCALE-scaled w_out, unscaled
    during the PSUM->SBUF copy.

Per-core dataflow:
  - x is fed pre-transposed (xT, [C, B*T]) so the C contraction sits on
    partitions.  qT/kT come out of the projection directly in [D, T] layout
    (D on partitions), v in natural [T, D] layout.
  - RoPE on DVE in [D, T] layout: rot_half is a partition-half swap done with
    two ACT copies, the sign folded into the sin table host-side.
  - scores computed transposed (sT[tk, tq] = kT.T @ qT); the causal mask
    (fully-masked columns AND the diagonal triangle, one composite constant
    sliced per diagonal position) is added on PE via an identity x mask
    matmul into the same PSUM bank, so every exp is one uniform full-width
    ACT op (masked entries underflow to zero) with 1/sqrt(D) folded in.
  - out_un[d, tq] accumulates v.T @ expT on PE; softmax denominators via a
    ones-column matmul over DVE-summed quads of e tiles.
  - normalization: fast approximate reciprocal of the colsum row on DVE,
    gpsimd partition-broadcast, one DVE multiply.
  - projection units are interleaved into the attention loops (PE backfill
    while ACT exp is the rate limiter) and stream to HBM as bf16 partials.
"""

import numpy as np
import ml_dtypes

import concourse.bass as bass
import concourse.tile as tile
import concourse.mybir as mybir
from concourse import bacc
from concourse.bass import ds
from concourse.bass_utils import run_bass_kernel_spmd

B, T, C, H, D = 2, 2048, 2048, 16, 128
NCORES = 8
HPC = H // NCORES  # heads per core = 2
S = B * T  # 4096 tokens
NBLK = T // 512  # 4 tq blocks of 512 per batch
NCT = C // 128  # 16 contraction tiles for the qkv projection
NTK = T // 128  # 16 tk tiles per batch
F32 = mybir.dt.float32
BF16 = mybir.dt.bfloat16
FP8 = mybir.dt.float8e4
WSCALE = 64.0  # w_qkv pre-scaled by this (fp8 denormal avoidance); undone in
               # the cos/sin tables (q,k) and the v-copy activation scale
EXP_SCALE = float(D) ** -0.5
NEG = -1.0e30

_CACHE = {}


def build_nc(reps=1):
    nc = bacc.Bacc("TRN2", target_bir_lowering=False, debug=False, num_devices=NCORES)

    xt_d = nc.dram_tensor("xt", [C, S], FP8, kind="ExternalInput").ap()
    xtb_d = nc.dram_tensor("xtb", [C, B * 512], BF16, kind="ExternalInput").ap()
    wqkv_d = nc.dram_tensor("wqkv", [C, 6 * D], FP8, kind="ExternalInput").ap()
    wqkvb_d = nc.dram_tensor("wqkvb", [C, 6 * D], BF16, kind="ExternalInput").ap()
    wout_d = nc.dram_tensor("wout", [HPC * D, C], BF16, kind="ExternalInput").ap()
    wout8_d = nc.dram_tensor("wout8", [HPC * D, C], FP8, kind="ExternalInput").ap()
    cos_d = nc.dram_tensor("cos2t", [D, T], F32, kind="ExternalInput").ap()
    sin_d = nc.dram_tensor("sin2t", [D, T], F32, kind="ExternalInput").ap()
    mask_d = nc.dram_tensor("maskadd", [128, 512], BF16, kind="ExternalInput").ap()
    ident_d = nc.dram_tensor("ident_in", [128, 128], BF16, kind="ExternalInput").ap()
    ones_d = nc.dram_tensor("ones_in", [128, 1], BF16, kind="ExternalInput").ap()
    y_d = nc.dram_tensor("y", [S, C], BF16, kind="ExternalOutput").ap()

    xt_t = xt_d.rearrange("(ct p) s -> p ct s", p=128)  # [128, 16, 4096]
    xtb_t = xtb_d.rearrange("(ct p) s -> p ct s", p=128)  # [128, 16, 1024]
    wqkv_t = wqkv_d.rearrange("(ct p) n -> p ct n", p=128)  # [128, 16, 768]
    wqkvb_t = wqkvb_d.rearrange("(ct p) n -> p ct n", p=128)  # [128, 16, 768]
    wout_t = wout_d.rearrange("(h p) n -> p h n", p=128)  # [128, 2, 2048]
    wout8_t = wout8_d.rearrange("(h p) n -> p h n", p=128)

    Exp = mybir.ActivationFunctionType.Exp
    Copy = mybir.ActivationFunctionType.Copy

    with tile.TileContext(nc) as tc:
        with (
            tc.tile_pool(name="cst", bufs=1) as cst,
            tc.tile_pool(name="sx", bufs=2) as sx,
            tc.tile_pool(name="sqk", bufs=1) as sqk,
            tc.tile_pool(name="srp", bufs=3) as srp,
            tc.tile_pool(name="se", bufs=6) as se,
            tc.tile_pool(name="snb", bufs=2) as snb,
            tc.tile_pool(name="sou", bufs=4) as sou,
            tc.tile_pool(name="sy", bufs=12) as sy,
            # PSUM: 8 banks total.
            tc.tile_pool(name="pqs", bufs=3, space="PSUM") as pqs,  # qkv ps + scores
            tc.tile_pool(name="pvc", bufs=1, space="PSUM") as pvc,  # v ps + colsum
            tc.tile_pool(name="pou", bufs=2, space="PSUM") as pou,  # out_un
            tc.tile_pool(name="py", bufs=2, space="PSUM") as py,  # proj out
        ):
            # ---- resident constants, ordered so the first qkv groups can
            # start ASAP: sync ring interleaves xt pieces with wqkv head
            # slices; scalar ring carries block-0 cos/sin then the rest.
            xt_first = sx.tile([128, NCT, 512], BF16, tag="xtb", name="xtb")
            wqkv = cst.tile([128, NCT, 6 * D], FP8, tag="wqkv", name="wqkv")
            wqkvb = cst.tile([128, NCT, 6 * D], BF16, tag="wqkvb", name="wqkvb")
            cos_sb = cst.tile([128, T], F32, tag="cos", name="cos_sb")
            sin_sb = cst.tile([128, T], F32, tag="sin", name="sin_sb")
            wout = cst.tile([128, HPC, C], BF16, tag="wout", name="wout")

            def _wq(ct, eng):
                eng.dma_start(wqkvb[:, ct, :], wqkvb_t[:, ct, :])

            def _xp(q, eng):
                eng.dma_start(
                    xt_first[:, ds(4 * q, 4), :], xtb_t[:, ds(4 * q, 4), ds(0, 512)]
                )

            _xp(0, nc.sync)
            for ct in range(1, NCT, 2):
                if ct == 9:
                    _xp(1, nc.sync)
                _wq(ct, nc.sync)
            for ct in range(0, NCT, 2):
                if ct == 8:
                    _xp(2, nc.scalar)
                elif ct == 12:
                    _xp(3, nc.scalar)
                _wq(ct, nc.scalar)
            nc.scalar.dma_start(cos_sb[:, ds(0, 512)], cos_d[:, ds(0, 512)])
            nc.scalar.dma_start(sin_sb[:, ds(0, 512)], sin_d[:, ds(0, 512)])
            nc.scalar.dma_start(wqkv[:], wqkv_t)  # fp8 weights for blocks 1-3
            nc.scalar.dma_start(cos_sb[:, ds(512, T - 512)], cos_d[:, ds(512, T - 512)])
            nc.scalar.dma_start(sin_sb[:, ds(512, T - 512)], sin_d[:, ds(512, T - 512)])
            nc.scalar.dma_start(wout[:], wout_t)
            wout8 = cst.tile([128, HPC, C], FP8, tag="wout8", name="wout8")
            nc.scalar.dma_start(wout8[:], wout8_t)
            mask = cst.tile([128, 512], BF16, tag="mask", name="mask")
            nc.gpsimd.dma_start(mask[:], mask_d)
            nbias = cst.tile([128, 1], F32, tag="nbias", name="nbias")
            nc.gpsimd.memset(nbias[:], -3.0)
            ident = cst.tile([128, 128], BF16, tag="ident", name="ident")
            nc.gpsimd.dma_start(ident[:], ident_d)
            ones = cst.tile([128, 1], BF16, tag="ones", name="ones")
            nc.gpsimd.dma_start(ones[:], ones_d)
            zeros = cst.tile([128, 512], BF16, tag="zeros", name="zeros")
            nc.gpsimd.memset(zeros[:], 0.0)

            def proj_unit(b, j, ou, tt, cb, k, alt=False):
                """One (tq tile, C block) unit of the output projection.

                ou is ('b', [ousb_h0, ousb_h1]) for the bf16 path (early
                tokens) or ('f', ou2) for the fp8 DoubleRow path, where ou2
                is [128(d), 2(head), 512(tq)] pre-scaled by WSCALE."""
                yps = py.tile([128, 512], F32, tag="y", name="yps")
                if ou[0] == "f":
                    nc.tensor.matmul(
                        yps[:],
                        ou[1][:, :, ds(tt * 128, 128)],
                        wout8[:, :, ds(cb * 512, 512)],
                        start=True,
                        stop=True,
                        perf_mode=mybir.MatmulPerfMode.DoubleRow,
                    )
                    unscale = 1.0 / (WSCALE * WSCALE)
                else:
                    for h in range(HPC):
                        nc.tensor.matmul(
                            yps[:],
                            ou[1][h][:, ds(tt * 128, 128)],
                            wout[:, h, ds(cb * 512, 512)],
                            start=(h == 0),
                            stop=(h == HPC - 1),
                        )
                    unscale = 1.0
                ysb = sy.tile([128, 512], BF16, tag="ysb", name="ysb")
                if (k % 2 == 1) if alt else (k % 3 == 2):  # ACT copy share
                    nc.scalar.activation(ysb[:], yps[:], Copy, scale=unscale)
                else:
                    # scalar_tensor_tensor, NOT tensor_copy/tensor_scalar:
                    # those enter DVE 2-port perf mode and lock the shared
                    # SBUF port against gpsimd (broadcast + SWDGE y-DMAs)
                    nc.vector.scalar_tensor_tensor(
                        ysb[:],
                        yps[:],
                        unscale,
                        zeros[:],
                        op0=mybir.AluOpType.mult,
                        op1=mybir.AluOpType.add,
                    )
                if alt:  # final block: both HWDGE rings (idle gpsimd drain)
                    yeng = nc.sync if k % 2 == 0 else nc.scalar
                else:
                    yeng = nc.sync if k % 2 == 0 else nc.gpsimd
                yeng.dma_start(
                    y_d[ds(b * T + j * 512 + tt * 128, 128), ds(cb * 512, 512)],
                    ysb[:],
                )

            UNITS = [(tt, cb) for tt in range(4) for cb in range(NBLK)]

            def proj_block(b, j, ou_sb, alt=False):
                """Project tq block j of batch b through w_out and DMA out."""
                for k, (tt, cb) in enumerate(UNITS):
                    proj_unit(b, j, ou_sb, tt, cb, k, alt=alt)

            pending = None  # (b, ou_sb) of the previous batch's last tq block

            for rep in range(reps):
             for b in range(B):
                # ================= qkv projection + RoPE =================
                qk = [
                    sqk.tile([128, T], BF16, tag=f"qk{i}", name=f"qk{i}")
                    for i in range(4)  # q0 q1 k0 k1
                ]
                v_sb = sqk.tile([128, 4, HPC * D], BF16, tag="v", name="v_sb")
                v_sb8 = sqk.tile([128, NTK, HPC * D], FP8, tag="v8", name="v_sb8")

                for blk in range(NBLK):  # 4 token blocks of 512
                    tok0 = b * T + blk * 512
                    bf_blk = blk == 0  # first block per batch: bf16 (accuracy)
                    if rep == 0 and b == 0 and blk == 0:
                        xt = xt_first
                    elif bf_blk:
                        xt = sx.tile([128, NCT, 512], BF16, tag="xtb", name="xtb")
                        nc.sync.dma_start(xt[:], xtb_t[:, :, ds(b * 512, 512)])
                    else:
                        xt = sx.tile([128, NCT, 512], FP8, tag="xt", name="xt")
                        nc.sync.dma_start(xt[:], xt_t[:, :, ds(tok0, 512)])

                    if blk == 1 and pending is not None:
                        # previous batch's last tq block projects here, after
                        # the first qkv group has covered its norm latency
                        proj_block(pending[0], NBLK - 1, pending[1])
                        pending = None

                    for ht in range(4):  # q0 q1 k0 k1
                        ps = pqs.tile([128, 512], F32, tag="m", name="qk_ps")
                        if bf_blk:
                            for ct in range(NCT):
                                nc.tensor.matmul(
                                    ps[:],
                                    wqkvb[:, ct, ds(ht * D, D)],
                                    xt[:, ct, :],
                                    start=(ct == 0),
                                    stop=(ct == NCT - 1),
                                )
                        else:
                            for cp in range(NCT // 2):  # fp8 DoubleRow ct-pairs
                                nc.tensor.matmul(
                                    ps[:],
                                    wqkv[:, ds(2 * cp, 2), ds(ht * D, D)],
                                    xt[:, ds(2 * cp, 2), :],
                                    start=(cp == 0),
                                    stop=(cp == NCT // 2 - 1),
                                    perf_mode=mybir.MatmulPerfMode.DoubleRow,
                                )
                        # RoPE: qk_blk = ps*cos + swap_halves(ps)*sin_signed
                        cs = cos_sb[:, ds(blk * 512, 512)]
                        sn = sin_sb[:, ds(blk * 512, 512)]
                        shuf = srp.tile([128, 512], F32, tag="shuf", name="shuf")
                        nc.scalar.copy(shuf[0:64, :], ps[64:128, :])
                        nc.scalar.copy(shuf[64:128, :], ps[0:64, :])
                        nc.vector.tensor_mul(shuf[:], shuf[:], sn)
                        tmp = srp.tile([128, 512], F32, tag="tmp", name="tmp")
                        nc.vector.tensor_mul(tmp[:], ps[:], cs)
                        nc.vector.tensor_add(
                            qk[ht][:, ds(blk * 512, 512)], tmp[:], shuf[:]
                        )

                    for half in range(2):  # 2 v psum tiles per block
                        vps = pvc.tile([128, 2, HPC * D], F32, tag="vc", name="v_ps")
                        for cch in range(2):
                            chunk = half * 2 + cch  # 128-token chunk in blk
                            if bf_blk:
                                for ct in range(NCT):
                                    nc.tensor.matmul(
                                        vps[:, cch, :],
                                        xt[:, ct, ds(chunk * 128, 128)],
                                        wqkvb[:, ct, ds(4 * D, HPC * D)],
                                        start=(ct == 0),
                                        stop=(ct == NCT - 1),
                                    )
                            else:
                                for cp in range(NCT // 2):
                                    nc.tensor.matmul(
                                        vps[:, cch, :],
                                        xt[:, ds(2 * cp, 2), ds(chunk * 128, 128)],
                                        wqkv[:, ds(2 * cp, 2), ds(4 * D, HPC * D)],
                                        start=(cp == 0),
                                        stop=(cp == NCT // 2 - 1),
                                        perf_mode=mybir.MatmulPerfMode.DoubleRow,
                                    )
                        i0 = blk * 4 + half * 2
                        nc.scalar.activation(
                            v_sb8[:, ds(i0, 2), :].rearrange("p a b -> p (a b)"),
                            vps[:].rearrange("p a b -> p (a b)"),
                            Copy,
                            scale=1.0 / WSCALE,
                        )
                        if i0 < 4:  # bf16 copy for the j=0 (early-token) path
                            nc.scalar.activation(
                                v_sb[:, ds(i0, 2), :].rearrange("p a b -> p (a b)"),
                                vps[:].rearrange("p a b -> p (a b)"),
                                Copy,
                                scale=1.0 / WSCALE,
                            )

                # ================= attention (+ inlined projection) ======
                prev_ou = None
                for j in range(NBLK):
                    ou_sb = []
                    if j > 0:
                        ou2 = sou.tile(
                            [128, HPC, 512], FP8, tag="ou2", name="ou2", bufs=2
                        )
                    units_left = list(enumerate(UNITS)) if j > 0 else []
                    for h in range(HPC):
                        qT, kT = qk[h], qk[2 + h]
                        ntk = 4 * j + 4
                        ou_ps = pou.tile([128, 512], F32, tag="ou", name="ou_ps")
                        cs_ps = pvc.tile([1, 512], F32, tag="vc", name="cs_ps")

                        def scores(i):
                            sp = pqs.tile([128, 512], F32, tag="m", name="sp")
                            rr = i - 4 * j
                            nc.tensor.matmul(
                                sp[:],
                                kT[:, ds(i * 128, 128)],
                                qT[:, ds(j * 512, 512)],
                                start=True,
                                stop=(rr < 0),
                            )
                            if rr >= 0:
                                # causal mask added on PE: the slice of the
                                # composite mask constant covers the fully
                                # masked columns AND the triangular square
                                w = (rr + 1) * 128
                                nc.tensor.matmul(
                                    sp[:, ds(0, w)],
                                    ident[:],
                                    mask[:, ds((3 - rr) * 128, w)],
                                    start=False,
                                    stop=True,
                                )
                            return sp

                        def exp_of(i, sp):
                            # masked scores carry -1e30 from the PE mask add,
                            # so a single full-width exp yields exact zeros
                            e = se.tile([128, 512], BF16, tag="e", name="e")
                            nc.scalar.activation(e[:], sp[:], Exp, scale=EXP_SCALE)
                            return e

                        DEPTH = 3
                        nquad = ntk // 4

                        def exp8(i, sp, dst):
                            # bias -3 keeps exp below fp8-e4m3's 448 ceiling;
                            # uniform within each softmax row, so the
                            # normalize ratio is unchanged (exact)
                            nc.scalar.activation(
                                dst, sp[:], Exp, scale=EXP_SCALE, bias=nbias[:]
                            )

                        if j > 0:
                            # fp8 e-pairs + DoubleRow out-matmuls: one PE
                            # instruction contracts two tk tiles (256 deep)
                            npair = ntk // 2
                            eps = []

                            def make_pair(ip):
                                ep = se.tile(
                                    [128, 2, 512], FP8, tag="ep", name="ep", bufs=4
                                )
                                exp8(2 * ip, scores(2 * ip), ep[:, 0, :])
                                exp8(2 * ip + 1, scores(2 * ip + 1), ep[:, 1, :])
                                return ep

                            eps.append(make_pair(0))
                            if npair > 1:
                                eps.append(make_pair(1))
                            last_e2 = None
                            for ip in range(npair):
                                if ip + 2 < npair:
                                    eps.append(make_pair(ip + 2))
                                ep = eps[ip]
                                nc.tensor.matmul(
                                    ou_ps[:],
                                    v_sb8[:, ds(2 * ip, 2), ds(h * D, D)],
                                    ep[:],
                                    start=(ip == 0),
                                    stop=(ip == npair - 1),
                                    perf_mode=mybir.MatmulPerfMode.DoubleRow,
                                )
                                if ip % 2 == 0:
                                    e2 = se.tile(
                                        [128, 512], BF16, tag="e2", name="e2", bufs=2
                                    )
                                    nc.vector.tensor_add(
                                        e2[:], ep[:, 0, :], ep[:, 1, :]
                                    )
                                    last_e2 = e2
                                else:
                                    e4 = se.tile(
                                        [128, 512], BF16, tag="e4", name="e4", bufs=2
                                    )
                                    nc.vector.tensor_add(
                                        e4[:], ep[:, 0, :], ep[:, 1, :]
                                    )
                                    nc.vector.tensor_add(e4[:], e4[:], last_e2[:])
                                    iq = ip // 2
                                    nc.tensor.matmul(
                                        cs_ps[:],
                                        ones[:],
                                        e4[:],
                                        start=(iq == 0),
                                        stop=(iq == nquad - 1),
                                    )
                                npop = 2 if ip < 2 else 1
                                for _ in range(npop):
                                    if units_left:
                                        k, (tt, cb) = units_left.pop(0)
                                        proj_unit(b, j - 1, prev_ou, tt, cb, k)
                        else:
                         es = []
                         for i in range(min(DEPTH, ntk)):
                            es.append(exp_of(i, scores(i)))
                         for i in range(ntk):
                            if i + DEPTH < ntk:
                                es.append(exp_of(i + DEPTH, scores(i + DEPTH)))
                            e = es[i]
                            nc.tensor.matmul(
                                ou_ps[:],
                                v_sb[:, i, ds(h * D, D)],
                                e[:],
                                start=(i == 0),
                                stop=(i == ntk - 1),
                            )
                            # colsum: sum quads of e tiles on DVE (bf16, 2x
                            # rate) so PE streams 1 colsum matmul per 4 tiles.
                            if i % 4 == 1:
                                e2 = se.tile(
                                    [128, 512], BF16, tag="e2", name="e2", bufs=2
                                )
                                nc.vector.tensor_add(e2[:], es[i - 1][:], e[:])
                                last_e2 = e2
                            elif i % 4 == 3:
                                e4 = se.tile(
                                    [128, 512], BF16, tag="e4", name="e4", bufs=2
                                )
                                nc.vector.tensor_add(e4[:], es[i - 1][:], e[:])
                                nc.vector.tensor_add(e4[:], e4[:], last_e2[:])
                                iq = i // 4
                                nc.tensor.matmul(
                                    cs_ps[:],
                                    ones[:],
                                    e4[:],
                                    start=(iq == 0),
                                    stop=(iq == nquad - 1),
                                )
                            # backfill PE with prev block's projection while
                            # ACT exp is the rate limiter in this loop (first
                            # unit only after out(0), so the previous norm
                            # chain latency hides under the exp(0) wait)
                            npop = 2 if i < 4 else (1 if i % 2 == 1 else 0)
                            for _ in range(npop):
                                if units_left:
                                    k, (tt, cb) = units_left.pop(0)
                                    proj_unit(b, j - 1, prev_ou, tt, cb, k)

                        row = snb.tile([1, 512], F32, tag="row", name="row")
                        nc.vector.reciprocal_approx_fast(row[:], cs_ps[:])
                        bc = snb.tile([128, 512], F32, tag="bc", name="bc")
                        nc.gpsimd.partition_broadcast(bc[:], row[0:1, :])
                        if j > 0:
                            # fp8 out tile, pre-scaled by WSCALE for the
                            # DoubleRow projection
                            nc.vector.scalar_tensor_tensor(
                                ou2[:, h, :],
                                ou_ps[:],
                                WSCALE,
                                bc[:],
                                op0=mybir.AluOpType.mult,
                                op1=mybir.AluOpType.mult,
                            )
                        else:
                            ousb = sou.tile(
                                [128, 512], BF16, tag="ou", name="ousb"
                            )
                            nc.vector.tensor_mul(ousb[:], ou_ps[:], bc[:])
                            ou_sb.append(ousb)

                    for k, (tt, cb) in units_left:  # flush remaining units
                        proj_unit(b, j - 1, prev_ou, tt, cb, k)
                    prev_ou = ("f", ou2) if j > 0 else ("b", ou_sb)
                pending = (b, prev_ou)
             if rep == reps - 1:
                proj_block(pending[0], NBLK - 1, pending[1], alt=True)
             # (non-final reps hand their last block to the next rep's qkv)

    nc.compile()
    return nc


def _host_prep(x, w_qkv, w_out, cos, sin):
    x = np.asarray(x, dtype=np.float32)
    w_qkv = np.asarray(w_qkv, dtype=np.float32)
    w_out = np.asarray(w_out, dtype=np.float32)
    cos = np.asarray(cos, dtype=np.float32)
    sin = np.asarray(sin, dtype=np.float32)

    bf16 = ml_dtypes.bfloat16
    fp8 = mybir.dt.np(FP8)
    xt_f = np.ascontiguousarray(x.reshape(S, C).T)  # [C, S]
    xt = xt_f.astype(fp8)
    # bf16 copy of the first 512-token block of each batch (see kernel doc)
    xtb = np.concatenate(
        [xt_f[:, b * T : b * T + 512] for b in range(B)], axis=1
    ).astype(bf16)
    # cos/sin tables carry the 1/WSCALE that undoes the w_qkv pre-scale
    cos2t = np.ascontiguousarray(np.concatenate([cos, cos], axis=1).T) / WSCALE
    sin2t = np.ascontiguousarray(np.concatenate([-sin, sin], axis=1).T) / WSCALE
    # composite causal mask: cols 0-383 fully masked, cols 384-511 the
    # strictly-lower-triangle square; slice [(3-rr)*128 : 512] serves every
    # diagonal tile position rr
    tri = np.tril(np.full((128, 128), NEG, dtype=np.float32), k=-1)
    maskadd = np.concatenate(
        [np.full((128, 384), NEG, dtype=np.float32), tri], axis=1
    ).astype(bf16)
    ident = np.eye(128, dtype=bf16)
    ones = np.ones((128, 1), dtype=bf16)

    in_maps = []
    for c in range(NCORES):
        h0 = c * HPC
        cols = []
        for qkv_i in range(3):
            for h in range(HPC):
                base = qkv_i * C + (h0 + h) * D
                cols.append(w_qkv[:, base : base + D])
        wqkv_f = np.concatenate(cols, axis=1) * WSCALE  # [C, 768]
        wqkv_c = wqkv_f.astype(fp8)
        wqkvb_c = wqkv_f.astype(bf16)
        wout_f = w_out[h0 * D : (h0 + HPC) * D, :]  # [256, C]
        wout_c = wout_f.astype(bf16)
        wout8_c = (wout_f * WSCALE).astype(fp8)
        in_maps.append(
            {
                "xt": xt,
                "xtb": xtb,
                "wqkvb": np.ascontiguousarray(wqkvb_c),
                "wqkv": np.ascontiguousarray(wqkv_c),
                "wout": np.ascontiguousarray(wout_c),
                "wout8": np.ascontiguousarray(wout8_c),
                "cos2t": cos2t,
                "sin2t": sin2t,
                "maskadd": maskadd,
                "ident_in": ident,
                "ones_in": ones,
            }
        )
    return in_maps


def _get_runner(reps=1):
    """Build (once) a jitted shard_map callable running the NEFF on 8 cores."""
    key = ("runner", reps)
    if key in _CACHE:
        return _CACHE[key]

    import jax
    from jax.sharding import Mesh, PartitionSpec
    try:
        from jax.experimental.shard_map import shard_map
    except ImportError:  # newer jax
        from jax.shard_map import shard_map  # type: ignore
    from concourse import bass2jax

    nckey = ("nc", reps)
    nc = _CACHE.get(nckey)
    if nc is None:
        nc = _CACHE[nckey] = build_nc(reps)
    bass2jax.install_neuronx_cc_hook()

    partition_name = (
        nc.partition_id_tensor.name if nc.partition_id_tensor else None
    )
    in_names, out_names, out_avals = [], [], []
    for alloc in nc.m.functions[0].allocations:
        if not isinstance(alloc, mybir.MemoryLocationSet):
            continue
        name = alloc.memorylocations[0].name
        if alloc.kind == "ExternalInput":
            if name != partition_name:
                in_names.append(name)
        elif alloc.kind == "ExternalOutput":
            out_names.append(name)
            out_avals.append(
                jax.core.ShapedArray(
                    tuple(alloc.tensor_shape), mybir.dt.np(alloc.dtype)
                )
            )
    n_params = len(in_names)
    all_names = in_names + out_names
    if partition_name is not None:
        all_names = all_names + [partition_name]

    def _body(*args):
        operands = list(args)
        if partition_name is not None:
            operands.append(bass2jax.partition_id_tensor())
        outs = bass2jax._bass_exec_p.bind(
            *operands,
            out_avals=tuple(out_avals),
            in_names=tuple(all_names),
            out_names=tuple(out_names),
            lowering_input_output_aliases=(),
            sim_require_finite=True,
            sim_require_nnan=True,
            nc=nc,
        )
        return tuple(outs)

    devices = jax.devices()[:NCORES]
    mesh = Mesh(np.asarray(devices), ("core",))
    nin = n_params + len(out_names)
    sharded = jax.jit(
        shard_map(
            _body,
            mesh=mesh,
            in_specs=(PartitionSpec("core"),) * nin,
            out_specs=(PartitionSpec("core"),) * len(out_names),
            check_rep=False,
        ),
        keep_unused=True,
    )
    zeros = [
        np.zeros((NCORES * a.shape[0], *a.shape[1:]), a.dtype) for a in out_avals
    ]
    _CACHE[key] = (sharded, in_names, out_names, out_avals, zeros, mesh)
    return _CACHE[key]


def _concat_inputs(in_maps, in_names):
    return [
        np.concatenate([m[nm] for m in in_maps], axis=0) for nm in in_names
    ]


def _run(in_maps):
    sharded, in_names, out_names, out_avals, zeros, mesh = _get_runner()
    concat_in = _concat_inputs(in_maps, in_names)
    out = sharded(*concat_in, *zeros)
    y = np.asarray(out[out_names.index("y")])
    return y.reshape(NCORES, S, C)


def kernel(x, w_qkv, w_out, cos, sin):
    in_maps = _host_prep(x, w_qkv, w_out, cos, sin)
    parts = _run(in_maps)
    acc = parts.astype(np.float32).sum(axis=0)
    return acc.reshape(B, T, C)


def time_exec(x, w_qkv, w_out, cos, sin, iters=10, reps=1):
    """Time device execution with device-resident inputs (excludes upload)."""
    import time as _time
    import jax

    sharded, in_names, out_names, out_avals, zeros, mesh = _get_runner(reps)
    in_maps = _host_prep(x, w_qkv, w_out, cos, sin)
    args = [jax.device_put(a) for a in _concat_inputs(in_maps, in_names)]
    zs = [jax.device_put(z) for z in zeros]
    out = sharded(*args, *zs)  # warm-up + compile
    jax.block_until_ready(out)
    times = []
    for _ in range(iters):
        t0 = _time.perf_counter()
        out = sharded(*args, *zs)
        jax.block_until_ready(out)
        times.append(_time.perf_counter() - t0)
    return times
